# revision 1
# baseline (speedup 1.0000x reference)
"""Trainium2 Bass kernel for nn_Loss_Synonymy.

reference:
    diff = S1 - S2                       # [B, 256]
    d    = sqrt(sum(diff^2, axis=-1))    # [B]
    t    = tanh(d)
    err  = where(score >= 0.8, relu(1 - t), relu(1 + t))
         = relu(1 + sign * t),  sign = -1 if score >= 0.8 else +1
    out  = sum(err) / B

Data-parallel over 8 NeuronCores: each core streams its 32768-row shard
(2 x 32 MiB) from HBM, computes a per-partition partial sum of err, and the
host combines the 8 x [128, 2] partials.

Per-core layout: tile t covers 2048 consecutive rows as [128 part, 4096 free]
(partition p holds rows t*2048 + p*16 + j for j in 0..15, i.e. 16 contiguous
rows = one contiguous 16 KiB stretch of HBM per partition; the whole tile is
one contiguous 2 MiB block -> near-peak DMA efficiency). The row-wise square-
sum is a free-dim reduction per 256-elem chunk: VectorE computes the diff,
then the 16 chunk-reductions are split between VectorE (scalar_tensor_tensor
with accum_out) and ScalarE (Square activation with accum_out) so neither
engine exceeds the ~190 us/core HBM streaming floor. The s1 stream rides the
SP HWDGE ring, the s2 stream rides SWDGE (gpsimd), keeping ScalarE free for
compute.
"""

import numpy as np

import concourse.bass as bass
import concourse.tile as tile
from concourse import bacc, mybir
from concourse.bass_utils import run_bass_kernel_spmd

F32 = mybir.dt.float32
AF = mybir.ActivationFunctionType
ALU = mybir.AluOpType

B = 262144
D = 256
NCORES = 8
BL = B // NCORES          # 32768 rows per core
J = 16                    # rows per partition per tile
TILE_ROWS = 128 * J       # 2048
NT = BL // TILE_ROWS      # 16 tiles
FREE = J * D              # 4096 (2 MiB per [128, FREE] f32 tile)
KD = 6                    # chunks handled by VectorE (scalar_tensor_tensor)
KA = J - KD               # chunks handled by ScalarE (Square + accum_out)
THRESH = 0.8
B_ENG = "sync"            # engine issuing the s2 stream DMAs (HWDGE ring)
BUFS_IN = 3               # double/triple buffering depth for input pools
BUFS_DIFF = 2

_NC_CACHE = {}


def _set_tile_rows(j, kd=None):
    """Recompute derived layout constants for rows-per-partition j."""
    global J, TILE_ROWS, NT, FREE, KD, KA
    J = j
    TILE_ROWS = 128 * J
    NT = BL // TILE_ROWS
    FREE = J * D
    if kd is None:
        kd = (J * 6) // 16
    KD = kd
    KA = J - KD


def _build_nc(reps=1):
    nc = bacc.Bacc(
        "TRN2", target_bir_lowering=False, debug=False, num_devices=NCORES
    )

    s1 = nc.dram_tensor("s1", [BL, D], F32, kind="ExternalInput").ap()
    s2 = nc.dram_tensor("s2", [BL, D], F32, kind="ExternalInput").ap()
    score = nc.dram_tensor("score", [BL], F32, kind="ExternalInput").ap()
    partial = nc.dram_tensor("partial", [128, 2], F32, kind="ExternalOutput").ap()

    # [NT, 128, J, D] views: tile t / partition p / row-chunk j / feature d
    s1_r = s1.rearrange("(t p j) d -> t p j d", t=NT, p=128, j=J)
    s2_r = s2.rearrange("(t p j) d -> t p j d", t=NT, p=128, j=J)
    score_r = score.rearrange("(t p j) -> p t j", t=NT, p=128, j=J)

    # Discarded elementwise outputs (only accum_out matters). Raw sbuf
    # tensors (not pool tiles) so Tile's release machinery ignores them.
    scr_dve = nc.alloc_sbuf_tensor("scr_dve", [128, D], F32).ap()
    scr_act = nc.alloc_sbuf_tensor("scr_act", [128, D], F32).ap()

    with tile.TileContext(nc) as tc:
        with (
            tc.tile_pool(name="in1", bufs=BUFS_IN) as p_in1,
            tc.tile_pool(name="in2", bufs=BUFS_IN) as p_in2,
            tc.tile_pool(name="diff", bufs=BUFS_DIFF) as p_diff,
            tc.tile_pool(name="persist", bufs=1) as p_per,
        ):
            sumsq_d = p_per.tile([128, NT * KD], F32, tag="sumsq_d")
            sumsq_a = p_per.tile([128, NT * KA], F32, tag="sumsq_a")
            score_d = p_per.tile([128, NT * KD], F32, tag="score_d")
            score_a = p_per.tile([128, NT * KA], F32, tag="score_a")
            part_sb = p_per.tile([128, 2], F32, tag="part_sb")

            # Small strided loads on SWDGE, once (scores don't change
            # across reps).
            nc.gpsimd.dma_start(
                score_d[:].rearrange("p (t j) -> p t j", j=KD),
                score_r[:, :, 0:KD],
            )
            nc.gpsimd.dma_start(
                score_a[:].rearrange("p (t j) -> p t j", j=KA),
                score_r[:, :, KD:J],
            )

            def body():
                for t in range(NT):
                    a = p_in1.tile([128, FREE], F32, tag="a")
                    nc.sync.dma_start(
                        a[:].rearrange("p (j d) -> p j d", d=D), s1_r[t]
                    )
                    b = p_in2.tile([128, FREE], F32, tag="b")
                    getattr(nc, B_ENG).dma_start(
                        b[:].rearrange("p (j d) -> p j d", d=D), s2_r[t]
                    )
                    diff = p_diff.tile([128, FREE], F32, tag="diff")
                    nc.vector.tensor_sub(diff[:], a[:], b[:])

                    for j in range(J):
                        chunk = diff[:, j * D : (j + 1) * D]
                        if j < KD:
                            c = t * KD + j
                            nc.vector.scalar_tensor_tensor(
                                scr_dve,
                                chunk,
                                1.0,
                                chunk,
                                ALU.mult,
                                ALU.mult,
                                accum_out=sumsq_d[:, c : c + 1],
                            )
                        else:
                            c = t * KA + (j - KD)
                            nc.scalar.activation(
                                scr_act,
                                chunk,
                                AF.Square,
                                accum_out=sumsq_a[:, c : c + 1],
                            )

                # Epilogue: err = relu(1 + sign*tanh(sqrt(sumsq)))
                for i, (ss, sc, w) in enumerate(
                    [(sumsq_d, score_d, NT * KD), (sumsq_a, score_a, NT * KA)]
                ):
                    dist = p_per.tile([128, w], F32, tag=f"dist{i}")
                    nc.scalar.activation(dist[:], ss[:], AF.Sqrt)
                    th = p_per.tile([128, w], F32, tag=f"th{i}")
                    nc.scalar.activation(th[:], dist[:], AF.Tanh)
                    # (score >= 0.8) * -2  ->  {-2, 0}
                    sgn = p_per.tile([128, w], F32, tag=f"sgn{i}")
                    nc.vector.tensor_scalar(
                        sgn[:], sc[:], THRESH, -2.0, ALU.is_ge, ALU.mult
                    )
                    # (sgn + 1) * th  ->  sign * tanh(d)
                    st = p_per.tile([128, w], F32, tag=f"st{i}")
                    nc.vector.scalar_tensor_tensor(
                        st[:], sgn[:], 1.0, th[:], ALU.add, ALU.mult
                    )
                    err = p_per.tile([128, w], F32, tag=f"err{i}")
                    nc.scalar.activation(
                        err[:],
                        st[:],
                        AF.Relu,
                        bias=1.0,
                        scale=1.0,
                        accum_out=part_sb[:, i : i + 1],
                    )

                nc.sync.dma_start(partial, part_sb[:])

            if reps == 1:
                body()
            else:
                with tc.For_i(0, reps, 1):
                    body()

    nc.compile()
    return nc


def _get_nc(reps=1):
    if reps not in _NC_CACHE:
        _NC_CACHE[reps] = _build_nc(reps)
    return _NC_CACHE[reps]


def make_in_maps(S1_out, S2_out, synonymy_score):
    in_maps = []
    for c in range(NCORES):
        lo, hi = c * BL, (c + 1) * BL
        in_maps.append(
            {
                "s1": np.ascontiguousarray(S1_out[lo:hi], dtype=np.float32),
                "s2": np.ascontiguousarray(S2_out[lo:hi], dtype=np.float32),
                "score": np.ascontiguousarray(
                    synonymy_score[lo:hi], dtype=np.float32
                ),
            }
        )
    return in_maps


def combine(results):
    total = np.float64(0.0)
    for r in results:
        total += r["partial"].astype(np.float64).sum()
    return np.asarray(total / B, dtype=np.float32)


def run(S1_out, S2_out, synonymy_score, trace=False, **trace_kwargs):
    nc = _get_nc()
    in_maps = make_in_maps(S1_out, S2_out, synonymy_score)
    res = run_bass_kernel_spmd(
        nc, in_maps, list(range(NCORES)), trace=trace, **trace_kwargs
    )
    return combine(res.results), res


def kernel(S1_out, S2_out, synonymy_score):
    out, _ = run(S1_out, S2_out, synonymy_score)
    return out



# revision 2
# speedup vs baseline: 1.0206x; 1.0206x over previous
"""Trainium2 Bass kernel for nn_Loss_Synonymy.

reference:
    diff = S1 - S2                       # [B, 256]
    d    = sqrt(sum(diff^2, axis=-1))    # [B]
    t    = tanh(d)
    err  = where(score >= 0.8, relu(1 - t), relu(1 + t))
    out  = sum(err) / B

Since tanh(d) in [0, 1) for d >= 0, relu(1 -+ tanh(d)) = 1 -+ tanh(d), so
err = 1 + sgn * tanh(d) with sgn = -1 (score >= 0.8) else +1, and
sum(err) = B + sum(sgn * tanh(d)).  The kernel only accumulates
sgn * tanh(d); the host adds B and divides.

Data-parallel over 8 NeuronCores, 32768 rows each.  Per-core layout:
partition p owns rows [p*256, (p+1)*256) of the shard, so the score
vector is ONE contiguous [128, 256] HWDGE load and the per-row sums
land as [128, 256] aligned with it.  s1/s2 are stacked host-side into
x[2, BL, D] so each tile is a single 4 MiB dma_start (2 x 16 KiB
contiguous per partition).  Tile t covers row-offsets [off, off+J) of
every partition's 256-row block:

    DMA  (sync HWDGE): X[128, 2*J*256] <- x[:, p*256+off : .. +J, :]
    DVE : diff = a - b           (in place, second half of X)
    ACT : sq   = Square(diff)    (in place)
    DVE : sumsq[:, off:off+J] = reduce_add(sq.view(128, J, 256), axis=X)

15 tiles of J=16 then 4 taper tiles of J=4 (shorter drain after the
last DMA).  Sqrt/Tanh activation tables are preloaded during the DMA
ramp.  Epilogue (single pass over [128, 256]):

    dist = Sqrt(sumsq); th = Tanh(dist)                  (ACT)
    sgn2 = (score >= 0.8) * -2                           (DVE)
    err  = (sgn2 + 1) * th, accum -> part[128, 1]        (DVE)

Host: out = (B + sum(partials)) / B.
"""

import numpy as np

import concourse.bass as bass
import concourse.tile as tile
from concourse import bacc, mybir
from concourse.bass_utils import run_bass_kernel_spmd

F32 = mybir.dt.float32
AF = mybir.ActivationFunctionType
ALU = mybir.AluOpType

B = 262144
D = 256
NCORES = 8
BL = B // NCORES          # 32768 rows per core
RPP = BL // 128           # 256 rows per partition
THRESH = 0.8

# (J, count): per-partition row-chunks per tile; sum(J*count) == RPP
TILING = [(16, 15), (4, 4)]
BUFS_BIG = 3
BUFS_SMALL = 4

_NC_CACHE = {}


def _build_nc():
    nc = bacc.Bacc(
        "TRN2", target_bir_lowering=False, debug=False, num_devices=NCORES
    )

    x = nc.dram_tensor("x", [2, BL, D], F32, kind="ExternalInput").ap()
    score = nc.dram_tensor("score", [BL], F32, kind="ExternalInput").ap()
    partial = nc.dram_tensor("partial", [128, 1], F32, kind="ExternalOutput").ap()

    # [128, 2, 256, 256]: partition p / source s / row-in-block c / feature d
    x_r = x.rearrange("s (p c) d -> p s c d", p=128, c=RPP)
    score_r = score.rearrange("(p c) -> p c", p=128, c=RPP)

    with tile.TileContext(nc) as tc:
        with (
            tc.tile_pool(name="big", bufs=BUFS_BIG) as p_big,
            tc.tile_pool(name="small", bufs=BUFS_SMALL) as p_small,
            tc.tile_pool(name="persist", bufs=1) as p_per,
        ):
            sumsq = p_per.tile([128, RPP], F32, tag="sumsq")
            score_sb = p_per.tile([128, RPP], F32, tag="score_sb")
            part_sb = p_per.tile([128, 1], F32, tag="part_sb")
            pre_scr = p_per.tile([128, 2], F32, tag="pre_scr")

            # Contiguous score load on the HWDGE ring (1 KiB per partition).
            nc.sync.dma_start(score_sb[:], score_r)

            # Preload Sqrt/Tanh activation tables while DMA ramps up, so
            # the epilogue doesn't pay the ~1.3 us ACT_TABLE_LOADs.
            nc.scalar.activation(pre_scr[:, 0:1], score_sb[:, 0:1], AF.Sqrt)
            nc.scalar.activation(pre_scr[:, 1:2], pre_scr[:, 0:1], AF.Tanh)

            off = 0
            for J, count in TILING:
                FREE = J * D
                pool = p_big if J == TILING[0][0] else p_small
                for _ in range(count):
                    X = pool.tile([128, 2 * FREE], F32, tag=f"x{J}")
                    nc.sync.dma_start(
                        X[:].rearrange("p (s j d) -> p s j d", s=2, d=D),
                        x_r[:, :, off : off + J, :],
                    )
                    a = X[:, 0:FREE]
                    b = X[:, FREE : 2 * FREE]
                    nc.vector.tensor_sub(b, a, b)
                    nc.scalar.activation(b, b, AF.Square)
                    nc.vector.tensor_reduce(
                        sumsq[:, off : off + J],
                        b.rearrange("p (j d) -> p j d", d=D),
                        axis=mybir.AxisListType.X,
                        op=ALU.add,
                    )
                    off += J

            # Epilogue: part = sum_p sgn * tanh(sqrt(sumsq))
            dist = p_per.tile([128, RPP], F32, tag="dist")
            nc.scalar.activation(dist[:], sumsq[:], AF.Sqrt)
            th = p_per.tile([128, RPP], F32, tag="th")
            nc.scalar.activation(th[:], dist[:], AF.Tanh)
            # (score >= 0.8) * -2  ->  {-2, 0}
            sgn2 = p_per.tile([128, RPP], F32, tag="sgn2")
            nc.vector.tensor_scalar(
                sgn2[:], score_sb[:], THRESH, -2.0, ALU.is_ge, ALU.mult
            )
            # (sgn2 + 1) * th -> +-tanh, accumulated per partition
            err = p_per.tile([128, RPP], F32, tag="err")
            nc.vector.scalar_tensor_tensor(
                err[:], sgn2[:], 1.0, th[:], ALU.add, ALU.mult,
                accum_out=part_sb[:],
            )

            nc.sync.dma_start(partial, part_sb[:])

    nc.compile()
    return nc


def _get_nc():
    if "nc" not in _NC_CACHE:
        _NC_CACHE["nc"] = _build_nc()
    return _NC_CACHE["nc"]


def make_in_maps(S1_out, S2_out, synonymy_score):
    in_maps = []
    for c in range(NCORES):
        lo, hi = c * BL, (c + 1) * BL
        x = np.empty((2, BL, D), dtype=np.float32)
        x[0] = S1_out[lo:hi]
        x[1] = S2_out[lo:hi]
        in_maps.append(
            {
                "x": x,
                "score": np.ascontiguousarray(
                    synonymy_score[lo:hi], dtype=np.float32
                ),
            }
        )
    return in_maps


def combine(results):
    total = np.float64(B)
    for r in results:
        total += r["partial"].astype(np.float64).sum()
    return np.asarray(total / B, dtype=np.float32)


def run(S1_out, S2_out, synonymy_score, trace=False, **trace_kwargs):
    nc = _get_nc()
    in_maps = make_in_maps(S1_out, S2_out, synonymy_score)
    res = run_bass_kernel_spmd(
        nc, in_maps, list(range(NCORES)), trace=trace, **trace_kwargs
    )
    return combine(res.results), res


def kernel(S1_out, S2_out, synonymy_score):
    out, _ = run(S1_out, S2_out, synonymy_score)
    return out
